# revision 34
# baseline (speedup 1.0000x reference)
"""DiT 2-block forward on 8 trn2 NeuronCores.

Strategy: sequence-parallel matmuls (each core owns 384 tokens, full weights,
channel-major activations) + head-parallel self-attention (2 heads/core) with
2 AllToAll exchanges per block. Cross-attention is fully local (replicated
ctx K/V computed during the qkv-exchange wait). LayerNorm affine/modulation
and all biases are folded into the weights host-side: the kernel only computes
per-token stats (mean*rsqrt) and applies them via one broadcast multiply on
the input plus one extra 2-row matmul per output tile, accumulated in PSUM.
Attention loops are software-pipelined so the PE never idles behind EXP.
"""
import numpy as np
import ml_dtypes

import concourse.bass as bass
import concourse.mybir as mybir
import concourse.tile as tile
from concourse import bacc
from concourse.bass_utils import run_bass_kernel_spmd

P = 128
L, D, H, HD, S, NB, DFF = 3072, 1024, 16, 64, 512, 2, 4096
NC = 8
LC = L // NC            # 384 tokens per core
DCH = D // P            # 8 din chunks
LCH = L // 512          # 6 l-chunks
MT = L // P             # 24 m-tiles (self)
KTC = S // P            # 4 k-tiles (cross)
FCH = DFF // P          # 32 dff chunks
dt = mybir.dt
AF = mybir.ActivationFunctionType
ALU = mybir.AluOpType
BF = ml_dtypes.bfloat16

# extra-weight (bias/mean-row) column offsets inside exw
EX_QK = 0                # 16 tiles x 128  (row0=-w1, row1=bW)
EX_V = EX_QK + 16 * P    # 2 groups x 512  (row0=-vw1, row1=bWv)
EX_SO = EX_V + 2 * 512   # 8 x 128         (row0=g*sob)
EX_CQ = EX_SO + DCH * P  # 8 x 128         (row0=-w1)
EX_CO = EX_CQ + DCH * P  # 8 x 128         (row0=cob)
EX_F1 = EX_CO + DCH * P  # 32 x 128        (row0=-w1, row1=bW+f1b)
EX_F2 = EX_F1 + FCH * P  # 8 x 128         (row0=g*f2b)
EX_COLS = EX_F2 + DCH * P

_cache = {}


def _build():
    nc = bacc.Bacc("TRN2", target_bir_lowering=False, debug=False,
                   enable_asserts=True, num_devices=NC)

    # ---------------- inputs ----------------
    x_t = nc.dram_tensor("x_t", [P, DCH * LC], dt.float32, kind="ExternalInput")
    ctx_t = nc.dram_tensor("ctx_t", [P, DCH * S], dt.bfloat16, kind="ExternalInput")
    cos2 = nc.dram_tensor("cos2", [P, L], dt.bfloat16, kind="ExternalInput")
    ss2 = nc.dram_tensor("ss2", [P, L], dt.bfloat16, kind="ExternalInput")
    permT = nc.dram_tensor("permT", [P, P], dt.bfloat16, kind="ExternalInput")
    exw = nc.dram_tensor("exw", [NB, 2, EX_COLS], dt.bfloat16, kind="ExternalInput")
    wqk = nc.dram_tensor("wqk", [NB, 16, P, DCH, P], dt.bfloat16, kind="ExternalInput")
    wv = nc.dram_tensor("wv", [NB, 2, P, DCH, 512], dt.bfloat16, kind="ExternalInput")
    wso = nc.dram_tensor("wso", [NB, DCH, P, DCH, P], dt.bfloat16, kind="ExternalInput")
    wcq = nc.dram_tensor("wcq", [NB, DCH, P, DCH, P], dt.bfloat16, kind="ExternalInput")
    wck = nc.dram_tensor("wck", [NB, DCH, P, DCH, P], dt.bfloat16, kind="ExternalInput")
    wcv = nc.dram_tensor("wcv", [NB, 2, P, DCH, 512], dt.bfloat16, kind="ExternalInput")
    wco = nc.dram_tensor("wco", [NB, DCH, P, DCH, P], dt.bfloat16, kind="ExternalInput")
    wf1 = nc.dram_tensor("wf1", [NB, FCH, P, DCH, P], dt.bfloat16, kind="ExternalInput")
    wf2 = nc.dram_tensor("wf2", [NB, DCH, 4, P, DCH, P], dt.bfloat16, kind="ExternalInput")
    gath = nc.dram_tensor("gath", [P, 256], dt.bfloat16, kind="ExternalInput")
    bcast = nc.dram_tensor("bcast", [16, DCH * P], dt.bfloat16, kind="ExternalInput")
    out_t = nc.dram_tensor("out_t", [P, DCH * LC], dt.float32, kind="ExternalOutput")

    RG = [list(range(NC))]

    from contextlib import ExitStack
    with tile.TileContext(nc) as tc, ExitStack() as ctx:
        cpool = ctx.enter_context(tc.tile_pool(name="cpool", bufs=1))
        spool = ctx.enter_context(tc.tile_pool(name="spool", bufs=1))
        wpool = ctx.enter_context(tc.tile_pool(name="wpool", bufs=6))
        wvpool = ctx.enter_context(tc.tile_pool(name="wvpool", bufs=2))
        ppool = ctx.enter_context(tc.tile_pool(name="ppool", bufs=6))
        opool = ctx.enter_context(tc.tile_pool(name="opool", bufs=2))
        bigp = ctx.enter_context(tc.tile_pool(name="bigp", bufs=1))
        stg = ctx.enter_context(tc.tile_pool(name="stg", bufs=2))
        stg1 = ctx.enter_context(tc.tile_pool(name="stg1", bufs=1))
        psA = ctx.enter_context(tc.tile_pool(name="psA", bufs=2, space="PSUM"))
        psB = ctx.enter_context(tc.tile_pool(name="psB", bufs=4, space="PSUM"))
        dram = ctx.enter_context(tc.tile_pool(name="dram", bufs=1, space="DRAM"))

        # ---------- persistent tiles ----------
        x_sb = cpool.tile([P, DCH, LC], dt.float32, tag="x_sb")
        nc.sync.dma_start(x_sb[:], x_t.ap().rearrange("p (o t) -> p o t", t=LC))
        ctxb = cpool.tile([P, DCH, S], dt.bfloat16, tag="ctxb")
        nc.sync.dma_start(ctxb[:], ctx_t.ap().rearrange("p (o t) -> p o t", t=S))
        cos_sb = cpool.tile([P, L], dt.bfloat16, tag="cos_sb")
        nc.sync.dma_start(cos_sb[:], cos2.ap())
        ss_sb = cpool.tile([P, L], dt.bfloat16, tag="ss_sb")
        nc.sync.dma_start(ss_sb[:], ss2.ap())
        pT_sb = cpool.tile([P, P], dt.bfloat16, tag="pT_sb")
        nc.sync.dma_start(pT_sb[:], permT.ap())
        ones1 = cpool.tile([P, 1], dt.bfloat16, tag="ones1")
        nc.gpsimd.memset(ones1[:], 1.0)
        onesr = cpool.tile([1, P], dt.bfloat16, tag="onesr")
        nc.gpsimd.memset(onesr[:], 1.0)
        onestok = cpool.tile([1, LC], dt.bfloat16, tag="onestok")
        nc.gpsimd.memset(onestok[:], 1.0)
        epsb = cpool.tile([P, 1], dt.float32, tag="epsb")
        nc.gpsimd.memset(epsb[:], 1e-6)
        gath_sb = cpool.tile([P, 256], dt.bfloat16, tag="gath_sb")
        nc.sync.dma_start(gath_sb[:], gath.ap())
        bc_sb = cpool.tile([16, DCH * P], dt.bfloat16, tag="bc_sb")
        nc.sync.dma_start(bc_sb[:], bcast.ap())

        # ---------- LN stats: rsb (rs bcast to 128p) + rhs2 ([mrs; ones]) + xr ----------
        def ln_begin(nm):
            return dict(
                nm=nm,
                xb=spool.tile([P, DCH, LC], dt.bfloat16, tag="xb", name=f"xb_{nm}"),
                ps1=psB.tile([P, 512], dt.float32, tag="psb", name=f"ps1_{nm}"),
                ps2=psB.tile([P, 512], dt.float32, tag="psb", name=f"ps2_{nm}"))

        def ln_chunk(st, o):
            xb = st['xb']
            if o % 2 == 0:
                nc.vector.tensor_copy(xb[:, o, :], x_sb[:, o, :])
            else:
                nc.scalar.activation(xb[:, o, :], x_sb[:, o, :], AF.Copy)
            xsq = stg.tile([P, LC], dt.bfloat16, tag="lnsq", name=f"xsq_{st['nm']}{o}")
            nc.vector.tensor_tensor(xsq[:], xb[:, o, :], xb[:, o, :], ALU.mult)
            nc.tensor.matmul(st['ps1'][:1, :LC], ones1[:], xb[:, o, :],
                             start=(o == 0), stop=(o == DCH - 1))
            nc.tensor.matmul(st['ps2'][:1, :LC], ones1[:], xsq[:],
                             start=(o == 0), stop=(o == DCH - 1))

        def ln_finish(st):
            xb, ps1, ps2 = st['xb'], st['ps1'], st['ps2']
            mrow = spool.tile([1, LC], dt.float32, tag="mrow")
            nc.vector.tensor_scalar_mul(mrow[:], ps1[:1, :LC], 1.0 / D)
            msq = spool.tile([1, LC], dt.float32, tag="msq")
            nc.vector.tensor_tensor(msq[:], mrow[:], mrow[:], ALU.mult)
            varr = spool.tile([1, LC], dt.float32, tag="varr")
            nc.vector.scalar_tensor_tensor(varr[:], ps2[:1, :LC], 1.0 / D, msq[:],
                                           ALU.mult, ALU.subtract)
            rs = spool.tile([1, LC], dt.float32, tag="rs")
            nc.scalar.activation(rs[:], varr[:], AF.Abs_reciprocal_sqrt, bias=epsb[:1])
            rsmb = spool.tile([1, LC], dt.bfloat16, tag="rsmb")
            nc.vector.tensor_copy(rsmb[:], rs[:])
            rhs2 = stg.tile([2, LC], dt.bfloat16, tag="rhs2")
            nc.gpsimd.memset(rhs2[:], 1.0)
            nc.vector.tensor_tensor(rhs2[0:1, :], mrow[:], rs[:], ALU.mult)
            # broadcast rs to 128 partitions via K=1 matmul
            psbc = psB.tile([P, 512], dt.float32, tag="psb")
            nc.tensor.matmul(psbc[:, 0:LC], onesr[:], rsmb[:], start=True, stop=True)
            rsb = stg.tile([P, LC], dt.bfloat16, tag="rsb")
            nc.vector.tensor_copy(rsb[:], psbc[:, 0:LC])
            xr = spool.tile([P, DCH, LC], dt.bfloat16, tag="hx",
                            name=f"xr_{st['nm']}")
            nc.vector.tensor_tensor(xr[:, 0:5, :], xb[:, 0:5, :],
                                    rsb[:, None, :].to_broadcast([P, 5, LC]), ALU.mult)
            nc.gpsimd.tensor_tensor(xr[:, 5:8, :], xb[:, 5:8, :],
                                    rsb[:, None, :].to_broadcast([P, 3, LC]), ALU.mult)
            return xr, rhs2

        def emit_ln(nm):
            st = ln_begin(nm)
            xb = st['xb']
            nc.vector.tensor_copy(xb[:, 0:5, :], x_sb[:, 0:5, :])
            nc.scalar.activation(xb[:, 5:8, :], x_sb[:, 5:8, :], AF.Copy)
            for o in range(DCH):
                nc.tensor.matmul(st['ps1'][:1, :LC], ones1[:], xb[:, o, :],
                                 start=(o == 0), stop=(o == DCH - 1))
            xsq = spool.tile([P, DCH, LC], dt.bfloat16, tag="lnscratch",
                             name=f"xsqf_{nm}")
            nc.vector.tensor_tensor(xsq[:], xb[:], xb[:], ALU.mult)
            for o in range(DCH):
                nc.tensor.matmul(st['ps2'][:1, :LC], ones1[:], xsq[:, o, :],
                                 start=(o == 0), stop=(o == DCH - 1))
            return ln_finish(st)

        # ---------- blocks ----------
        for i in range(NB):
            def exload(off, n):  # load [2, n] extra-weight slice for this phase
                exch = stg1.tile([2, FCH * P], dt.bfloat16, tag="exch")
                nc.sync.dma_start(exch[:, 0:n], exw.ap()[i, :, off:off + n])
                return exch

            # ===== AdaLN + self-attention =====
            xr, rhs2 = emit_ln(f"ln1b{i}")

            qkv_in = dram.tile([NC, 2 * P * LC], dt.bfloat16, tag="qkv_in")
            qkv_out = dram.tile([NC, 2 * P * LC], dt.bfloat16, tag="qkv_out")
            v_in = dram.tile([NC, P * LC], dt.bfloat16, tag="v_in")
            v_out = dram.tile([NC, P * LC], dt.bfloat16, tag="v_out")
            # q, k projections (channel-major lhsT tiles) -> staged to DRAM
            stq = stg1.tile([P, DCH, LC], dt.bfloat16, tag="stq")
            stk = stg1.tile([P, DCH, LC], dt.bfloat16, tag="stk")
            exqk = exload(EX_QK, 16 * P)
            for j in range(16):
                wt = wpool.tile([P, DCH, P], dt.bfloat16, tag="w8")
                nc.sync.dma_start(wt[:], wqk.ap()[i, j])
                pp = psB.tile([P, 512], dt.float32, tag="psb")
                for o in range(DCH):
                    nc.tensor.matmul(pp[:, :LC], wt[:, o, :], xr[:, o, :],
                                     start=(o == 0), stop=False)
                nc.tensor.matmul(pp[:, :LC], exqk[:, j * P:(j + 1) * P], rhs2[:],
                                 start=False, stop=True)
                dst = (stq if j < 8 else stk)[:, j % 8, :]
                if j % 2 == 0:
                    nc.vector.tensor_copy(dst, pp[:, :LC])
                else:
                    nc.scalar.activation(dst, pp[:, :LC], AF.Copy)
            nc.sync.dma_start(
                qkv_in[:, 0:P * LC].rearrange("s (p t) -> p s t", p=P), stq[:])
            nc.sync.dma_start(
                qkv_in[:, P * LC:2 * P * LC].rearrange("s (p t) -> p s t", p=P), stk[:])
            nc.gpsimd.collective_compute("AllToAll", ALU.bypass, replica_groups=RG,
                                         ins=[qkv_in.opt()], outs=[qkv_out.opt()])
            # v projection (token-major)
            stv = stg1.tile([P, 2, 3, 512], dt.bfloat16, tag="stv")
            exv = exload(EX_V, 2 * 512)
            for g in range(2):
                wvt = wvpool.tile([P, DCH, 512], dt.bfloat16, tag="wv512")
                nc.sync.dma_start(wvt[:], wv.ap()[i, g])
                for tcix in range(LC // P):
                    pp = psB.tile([P, 512], dt.float32, tag="psb")
                    for o in range(DCH):
                        nc.tensor.matmul(pp[:], xr[:, o, tcix * P:(tcix + 1) * P],
                                         wvt[:, o, :], start=(o == 0), stop=False)
                    nc.tensor.matmul(pp[:], rhs2[:, tcix * P:(tcix + 1) * P],
                                     exv[:, g * 512:(g + 1) * 512],
                                     start=False, stop=True)
                    if tcix % 2 == 0:
                        nc.vector.tensor_copy(stv[:, g, tcix, :], pp[:])
                    else:
                        nc.scalar.activation(stv[:, g, tcix, :], pp[:], AF.Copy)
            for g in range(2):
                for s4 in range(4):
                    dstv = v_in[4 * g + s4, :] \
                        .rearrange("(tc p c) -> p tc c", p=P, c=P)
                    nc.sync.dma_start(dstv, stv[:, g, :, s4 * P:(s4 + 1) * P])
            nc.gpsimd.collective_compute("AllToAll", ALU.bypass, replica_groups=RG,
                                         ins=[v_in.opt()], outs=[v_out.opt()])

            # ===== cross K/V over all ctx tokens (fills the AllToAll wait) =====
            k2c = bigp.tile([P, DCH, S], dt.bfloat16, tag="k2c")

            def emit_crossk(jlist):
                for j in jlist:
                    wt = wpool.tile([P, DCH, P], dt.bfloat16, tag="w8", name=f"wck{j}")
                    nc.sync.dma_start(wt[:], wck.ap()[i, j])
                    psk = psB.tile([P, 512], dt.float32, tag="psb", name=f"psk{j}")
                    for o in range(DCH):
                        nc.tensor.matmul(psk[:, :S], wt[:, o, :], ctxb[:, o, :],
                                         start=(o == 0), stop=(o == DCH - 1))
                    if j % 2 == 0:
                        nc.vector.tensor_copy(k2c[:, j, :], psk[:, :S])
                    else:
                        nc.scalar.activation(k2c[:, j, :], psk[:, :S], AF.Copy)

            emit_crossk(range(0, 4))
            # cross-V is emitted as PE filler work inside the flash loop below,
            # to keep the PE saturated (and the HAM clock warm) while EXP gates.
            vextc = bigp.tile([P, KTC, DCH, 130], dt.bfloat16, tag="vextc")
            nc.gpsimd.memset(vextc[:, :, :, 64:65], 1.0)
            nc.gpsimd.memset(vextc[:, :, :, 129:130], 1.0)
            fillers = []
            cvstate = {}

            def _cv_load(half):
                wcvt = wvpool.tile([P, DCH, 512], dt.bfloat16, tag="wv512",
                                   name=f"wcvt{half}")
                nc.sync.dma_start(wcvt[:], wcv.ap()[i, half])
                cvstate['w'] = wcvt

            def _cv_mm(half, kt, o):
                if o == 0:
                    cvstate['psv'] = psB.tile([P, 512], dt.float32, tag="psb",
                                              name=f"psv{half}_{kt}")
                nc.tensor.matmul(cvstate['psv'][:], ctxb[:, o, kt * P:(kt + 1) * P],
                                 cvstate['w'][:, o, :],
                                 start=(o == 0), stop=(o == DCH - 1))

            def _cv_copy(half, kt):
                pv3 = cvstate['psv'][:].rearrange("p (j c) -> p j c", c=P)
                jj = slice(4 * half, 4 * half + 4)
                nc.vector.tensor_copy(vextc[:, kt, jj, 0:64], pv3[:, :, 0:64])
                nc.vector.tensor_copy(vextc[:, kt, jj, 65:129], pv3[:, :, 64:128])

            from functools import partial
            for half in range(2):
                fillers.append(partial(_cv_load, half))
                for kt in range(KTC):
                    for o in range(DCH):
                        fillers.append(partial(_cv_mm, half, kt, o))
                    fillers.append(partial(_cv_copy, half, kt))
            fillers.reverse()  # pop() from the front

            # ===== unpack qkv exchange =====
            q_sb = bigp.tile([P, L], dt.bfloat16, tag="q_sb")
            k_sb = bigp.tile([P, L], dt.bfloat16, tag="k_sb")
            nc.sync.dma_start(q_sb[:].rearrange("p (s t) -> p s t", t=LC),
                              qkv_out[:, 0:P * LC].rearrange("s (p t) -> p s t", p=P))
            nc.sync.dma_start(k_sb[:].rearrange("p (s t) -> p s t", t=LC),
                              qkv_out[:, P * LC:2 * P * LC].rearrange(
                                  "s (p t) -> p s t", p=P))
            vext = bigp.tile([P, MT, 130], dt.bfloat16, tag="vext")
            nc.gpsimd.memset(vext[:, :, 64:65], 1.0)
            nc.gpsimd.memset(vext[:, :, 129:130], 1.0)
            for s in range(NC):
                vsrc = v_out[s, :].rearrange("(tc p c) -> p tc c", p=P, c=P)
                nc.sync.dma_start(vext[:, s * 3:(s + 1) * 3, 0:64], vsrc[:, :, 0:64])
                nc.sync.dma_start(vext[:, s * 3:(s + 1) * 3, 65:129],
                                  vsrc[:, :, 64:128])

            # RoPE (perm matmul + combine, in-place on q_sb/k_sb; k first)
            for src in (k_sb, q_sb):
                for lc in range(LCH):
                    sl = slice(lc * 512, (lc + 1) * 512)
                    psr = psB.tile([P, 512], dt.float32, tag="psb")
                    nc.tensor.matmul(psr[:], pT_sb[:], src[:, sl], start=True, stop=True)
                    rt = stg.tile([P, 512], dt.bfloat16, tag="rtmp")
                    nc.vector.tensor_tensor(rt[:], psr[:], ss_sb[:, sl], ALU.mult)
                    nc.gpsimd.tensor_tensor(src[:, sl], src[:, sl], cos_sb[:, sl],
                                            ALU.mult)
                    nc.vector.tensor_tensor(src[:, sl], src[:, sl], rt[:], ALU.add)
            qr, kr = q_sb, k_sb

            # flash attention, software-pipelined: scores(mt+1) issued before PV(mt)
            o_h0 = opool.tile([65, L], dt.bfloat16, tag="osb")
            o_h1 = opool.tile([65, L], dt.bfloat16, tag="osb")
            for lc in range(LCH):
                sl = slice(lc * 512, (lc + 1) * 512)
                pso0 = psB.tile([P, 512], dt.float32, tag="psb")
                pso1 = psB.tile([P, 512], dt.float32, tag="psb")

                def sc_mm(mt):
                    pqk = psA.tile([P, 1024], dt.float32, tag="psa")
                    nc.tensor.matmul(pqk[:, 0:512], kr[0:64, mt * P:(mt + 1) * P],
                                     qr[0:64, sl], start=True, stop=True)
                    nc.tensor.matmul(pqk[:, 512:1024], kr[64:128, mt * P:(mt + 1) * P],
                                     qr[64:128, sl], start=True, stop=True)
                    return pqk

                pqk_cur = sc_mm(0)
                for mt in range(MT):
                    Pt = ppool.tile([P, 1024], dt.bfloat16, tag="Pt")
                    nc.scalar.activation(Pt[:], pqk_cur[:], AF.Exp, scale=HD ** -0.5)
                    if mt + 1 < MT:
                        pqk_cur = sc_mm(mt + 1)
                    nc.tensor.matmul(pso0[:65, :], vext[:, mt, 0:65], Pt[:, 0:512],
                                     start=(mt == 0), stop=(mt == MT - 1))
                    nc.tensor.matmul(pso1[:65, :], vext[:, mt, 65:130], Pt[:, 512:1024],
                                     start=(mt == 0), stop=(mt == MT - 1))
                    if fillers:
                        fillers.pop()()
                nc.vector.tensor_copy(o_h0[:, sl], pso0[:65, :])
                nc.vector.tensor_copy(o_h1[:, sl], pso1[:65, :])
            while fillers:
                fillers.pop()()

            # ===== o exchange + o-proj + residual =====
            oa_in = dram.tile([NC, 2, 65, LC], dt.bfloat16, tag="oa_in")
            oa_out = dram.tile([NC, 2, 65, LC], dt.bfloat16, tag="oa_out")
            nc.sync.dma_start(oa_in[:, 0].rearrange("s c t -> c s t"),
                              o_h0[:].rearrange("c (s t) -> c s t", t=LC))
            nc.sync.dma_start(oa_in[:, 1].rearrange("s c t -> c s t"),
                              o_h1[:].rearrange("c (s t) -> c s t", t=LC))
            nc.gpsimd.collective_compute("AllToAll", ALU.bypass, replica_groups=RG,
                                         ins=[oa_in.opt()], outs=[oa_out.opt()])
            emit_crossk(range(4, DCH))

            orecv = spool.tile([P, DCH, LC], dt.bfloat16, tag="xb")
            nc.sync.dma_start(orecv[0:64, :, :],
                              oa_out[:, 0, 0:64, :].rearrange("s c t -> c s t"))
            nc.sync.dma_start(orecv[64:128, :, :],
                              oa_out[:, 1, 0:64, :].rearrange("s c t -> c s t"))
            sums = spool.tile([16, LC], dt.bfloat16, tag="sums")
            nc.sync.dma_start(sums[:], oa_out[:, :, 64, :])
            rcp = spool.tile([16, LC], dt.float32, tag="rcp")
            nc.vector.reciprocal(rcp[:], sums[:])
            rcpb = spool.tile([16, LC], dt.bfloat16, tag="rcpb")
            nc.vector.tensor_copy(rcpb[:], rcp[:])
            exso = exload(EX_SO, DCH * P)
            for j in range(DCH):
                psbj = psB.tile([P, 512], dt.float32, tag="psb")
                nc.tensor.matmul(psbj[:, 0:LC], bc_sb[:, j * P:(j + 1) * P], rcpb[:],
                                 start=True, stop=True)
                nc.vector.tensor_tensor(orecv[:, j, :], orecv[:, j, :],
                                        psbj[:, 0:LC], ALU.mult)
            for j in range(DCH):
                wt = wpool.tile([P, DCH, P], dt.bfloat16, tag="w8")
                nc.sync.dma_start(wt[:], wso.ap()[i, j])
                pp = psB.tile([P, 512], dt.float32, tag="psb")
                for o in range(DCH):
                    nc.tensor.matmul(pp[:, :LC], wt[:, o, :], orecv[:, o, :],
                                     start=(o == 0), stop=False)
                nc.tensor.matmul(pp[:, :LC], exso[0:1, j * P:(j + 1) * P], onestok[:],
                                 start=False, stop=True)
                nc.vector.scalar_tensor_tensor(x_sb[:, j, :], pp[:, :LC], 0.0,
                                               x_sb[:, j, :], ALU.bypass, ALU.add)

            # ===== cross-attention (fully local) =====
            xr2, rhs2c = emit_ln(f"ln2b{i}")
            cq_sb = spool.tile([P, DCH, LC], dt.bfloat16, tag="xb")
            excq = exload(EX_CQ, DCH * P)

            def emit_cq(j):
                wt = wpool.tile([P, DCH, P], dt.bfloat16, tag="w8", name=f"wcq{j}")
                nc.sync.dma_start(wt[:], wcq.ap()[i, j])
                pp = psB.tile([P, 512], dt.float32, tag="psb", name=f"cqpp{j}")
                for o in range(DCH):
                    nc.tensor.matmul(pp[:, :LC], wt[:, o, :], xr2[:, o, :],
                                     start=(o == 0), stop=False)
                nc.tensor.matmul(pp[:, :LC], excq[0:1, j * P:(j + 1) * P],
                                 rhs2c[0:1, :], start=False, stop=True)
                nc.vector.tensor_copy(cq_sb[:, j, :], pp[:, :LC])

            emit_cq(0)
            emit_cq(1)

            o_all = spool.tile([P, DCH, LC], dt.bfloat16, tag="lnscratch")
            oe65 = stg1.tile([P, DCH, LC], dt.bfloat16, tag="stk")
            oo65 = stg1.tile([P, DCH, LC], dt.bfloat16, tag="stq")
            psden = psB.tile([P, 512], dt.float32, tag="psb")
            for j in range(DCH):
                psca = psB.tile([P, 512], dt.float32, tag="psb")
                pscb = psB.tile([P, 512], dt.float32, tag="psb")

                def csc_mm(kt):
                    pqc = psA.tile([P, 1024], dt.float32, tag="psa")
                    nc.tensor.matmul(pqc[:, 0:LC], k2c[0:64, j, kt * P:(kt + 1) * P],
                                     cq_sb[0:64, j, :], start=True, stop=True)
                    nc.tensor.matmul(pqc[:, 512:512 + LC],
                                     k2c[64:128, j, kt * P:(kt + 1) * P],
                                     cq_sb[64:128, j, :], start=True, stop=True)
                    return pqc

                pqc_cur = csc_mm(0)
                if j + 2 < DCH:
                    emit_cq(j + 2)
                for kt in range(KTC):
                    Ptc = ppool.tile([P, 1024], dt.bfloat16, tag="Pt")
                    nc.scalar.activation(
                        Ptc[:, 0:2 * LC].rearrange("p (b c) -> p b c", c=LC),
                        pqc_cur[:].rearrange("p (b c) -> p b c", c=512)[:, :, 0:LC],
                        AF.Exp, scale=HD ** -0.5)
                    if kt + 1 < KTC:
                        pqc_cur = csc_mm(kt + 1)
                    nc.tensor.matmul(psca[:65, :LC], vextc[:, kt, j, 0:65],
                                     Ptc[:, 0:LC], start=(kt == 0), stop=(kt == KTC - 1))
                    nc.tensor.matmul(pscb[:65, :LC], vextc[:, kt, j, 65:130],
                                     Ptc[:, LC:2 * LC], start=(kt == 0),
                                     stop=(kt == KTC - 1))
                nc.vector.tensor_copy(oe65[0:65, j, :], psca[0:65, :LC])
                nc.scalar.activation(oo65[0:65, j, :], pscb[0:65, :LC], AF.Copy)
                # gather the two denominator rows (row 64) into psden rows 2j/2j+1
                nc.tensor.matmul(psden[:16, :LC], gath_sb[0:65, 32 * j:32 * j + 16],
                                 oe65[0:65, j, :], start=(j == 0), stop=False)
                nc.tensor.matmul(psden[:16, :LC], gath_sb[0:65, 32 * j + 16:32 * j + 32],
                                 oo65[0:65, j, :], start=False, stop=(j == DCH - 1))
            # assemble channel-major o via SBUF->SBUF DMAs
            nc.sync.dma_start(o_all[0:64, :, :], oe65[0:64, :, :])
            nc.sync.dma_start(o_all[64:128, :, :], oo65[0:64, :, :])
            rcpc = spool.tile([16, LC], dt.float32, tag="rcp")
            nc.vector.reciprocal(rcpc[:], psden[:16, :LC])
            rcpcb = spool.tile([16, LC], dt.bfloat16, tag="rcpb")
            nc.vector.tensor_copy(rcpcb[:], rcpc[:])
            for j in range(DCH):
                psbj = psB.tile([P, 512], dt.float32, tag="psb")
                nc.tensor.matmul(psbj[:, 0:LC], bc_sb[:, j * P:(j + 1) * P], rcpcb[:],
                                 start=True, stop=True)
                nc.vector.tensor_tensor(o_all[:, j, :], o_all[:, j, :],
                                        psbj[:, 0:LC], ALU.mult)
            exco = exload(EX_CO, DCH * P)
            for j in range(DCH):
                wt = wpool.tile([P, DCH, P], dt.bfloat16, tag="w8")
                nc.sync.dma_start(wt[:], wco.ap()[i, j])
                pp = psB.tile([P, 512], dt.float32, tag="psb")
                for o in range(DCH):
                    nc.tensor.matmul(pp[:, :LC], wt[:, o, :], o_all[:, o, :],
                                     start=(o == 0), stop=False)
                nc.tensor.matmul(pp[:, :LC], exco[0:1, j * P:(j + 1) * P], onestok[:],
                                 start=False, stop=True)
                nc.vector.scalar_tensor_tensor(x_sb[:, j, :], pp[:, :LC], 0.0,
                                               x_sb[:, j, :], ALU.bypass, ALU.add)

            # ===== AdaLN + MLP =====
            xr3, rhs2m = emit_ln(f"ln3b{i}")
            g_sb = cpool.tile([P, FCH, LC], dt.bfloat16, tag="g_sb")
            exf1 = exload(EX_F1, FCH * P)
            for j in range(FCH):
                wt = wpool.tile([P, DCH, P], dt.bfloat16, tag="w8")
                nc.sync.dma_start(wt[:], wf1.ap()[i, j])
                pp = psB.tile([P, 512], dt.float32, tag="psb")
                for o in range(DCH):
                    nc.tensor.matmul(pp[:, :LC], wt[:, o, :], xr3[:, o, :],
                                     start=(o == 0), stop=False)
                nc.tensor.matmul(pp[:, :LC], exf1[:, j * P:(j + 1) * P],
                                 rhs2m[:], start=False, stop=True)
                nc.scalar.activation(g_sb[:, j, :], pp[:, :LC], AF.Gelu)
            exf2 = exload(EX_F2, DCH * P)
            for j in range(DCH):
                pp = psB.tile([P, 512], dt.float32, tag="psb")
                for og in range(4):
                    wt = wpool.tile([P, DCH, P], dt.bfloat16, tag="w8")
                    nc.sync.dma_start(wt[:], wf2.ap()[i, j, og])
                    for o2 in range(DCH):
                        nc.tensor.matmul(pp[:, :LC], wt[:, o2, :], g_sb[:, og * DCH + o2, :],
                                         start=(og == 0 and o2 == 0), stop=False)
                nc.tensor.matmul(pp[:, :LC], exf2[0:1, j * P:(j + 1) * P], onestok[:],
                                 start=False, stop=True)
                nc.vector.scalar_tensor_tensor(x_sb[:, j, :], pp[:, :LC], 0.0,
                                               x_sb[:, j, :], ALU.bypass, ALU.add)

        nc.sync.dma_start(out_t.ap().rearrange("p (o t) -> p o t", t=LC), x_sb[:])

    nc.compile()
    return nc


def _host_prep(inputs):
    """Build per-core in_maps from full inputs (LN affine folded into weights)."""
    f32 = np.float32
    x = np.asarray(inputs["x"], f32)[0]           # [L, D]
    te = np.asarray(inputs["timestep_emb"], f32)  # [1, D]
    ctx = np.asarray(inputs["context_emb"], f32)[0]
    rope = np.asarray(inputs["rope_emb"], f32)    # [L, HD]
    cos, sin = np.cos(rope), np.sin(rope)

    def sbufize(a2d):  # [D, T] -> [128, DCH*T] channel-major sbuf layout
        Dd, T = a2d.shape
        return np.ascontiguousarray(
            a2d.reshape(Dd // P, P, T).transpose(1, 0, 2).reshape(P, (Dd // P) * T))

    def lhst5(WT, jn):  # WT [D, DOUT] -> [jn, P, DCH, P] tiles of W^T
        Dd, DO = WT.shape
        a = WT.reshape(DCH, P, jn, P)         # [o, p, j, f]
        return np.ascontiguousarray(a.transpose(2, 1, 0, 3)).astype(BF)  # [j, p, o, f]

    cos2 = np.tile(cos.T, (2, 1)).astype(BF)                      # [128, L]
    ssg = np.concatenate([-sin.T[:32], sin.T[32:]], 0)
    ss2 = np.tile(ssg, (2, 1)).astype(BF)
    sig = (np.arange(P) + 32) % 64 + 64 * (np.arange(P) // 64)
    permT = np.zeros((P, P), f32)
    permT[sig, np.arange(P)] = 1.0

    wqk = np.zeros((NB, 16, P, DCH, P), BF)
    wvv = np.zeros((NB, 2, P, DCH, 512), BF)
    wso = np.zeros((NB, DCH, P, DCH, P), BF)
    wcq = np.zeros((NB, DCH, P, DCH, P), BF)
    wck_f = np.zeros((NB, DCH, P, DCH, P), BF)
    wcv_f = np.zeros((NB, 2, P, DCH, 512), BF)
    wco = np.zeros((NB, DCH, P, DCH, P), BF)
    wf1 = np.zeros((NB, FCH, P, DCH, P), BF)
    wf2 = np.zeros((NB, DCH, 4, P, DCH, P), BF)
    exw_np = np.zeros((NB, 2, EX_COLS), f32)

    for i in range(NB):
        mods = (te @ np.asarray(inputs["adaW"], f32)[i].T
                + np.asarray(inputs["adab"], f32)[i])[0]
        sh_msa, sc_msa, g_msa, sh_mlp, sc_mlp, g_mlp = np.split(mods, 6)
        A_msa, A_mlp = 1.0 + sc_msa, 1.0 + sc_mlp
        sob = np.asarray(inputs["sob"], f32)[i]
        cob = np.asarray(inputs["cob"], f32)[i]
        f1b = np.asarray(inputs["f1b"], f32)[i]
        f2b = np.asarray(inputs["f2b"], f32)[i]

        sq, sk, sv = (np.asarray(inputs[k], f32)[i] for k in ("sqW", "skW", "svW"))
        # folded transposed weights [din, dout]
        sqT = sq.T * A_msa[:, None]
        skT = sk.T * A_msa[:, None]
        svT = sv.T * A_msa[:, None]
        wqk[i, :8] = lhst5(sqT, 8)
        wqk[i, 8:] = lhst5(skT, 8)
        wvv[i] = np.ascontiguousarray(
            svT.reshape(DCH, P, 2, 512).transpose(2, 1, 0, 3)).astype(BF)
        # extra rows: qk (row0=-colsum, row1=sh@W.T)
        exw_np[i, 0, EX_QK:EX_QK + 8 * P] = -sqT.sum(0)
        exw_np[i, 0, EX_QK + 8 * P:EX_QK + 16 * P] = -skT.sum(0)
        exw_np[i, 1, EX_QK:EX_QK + 8 * P] = sh_msa @ sq.T
        exw_np[i, 1, EX_QK + 8 * P:EX_QK + 16 * P] = sh_msa @ sk.T
        exw_np[i, 0, EX_V:EX_V + 1024] = -svT.sum(0)
        exw_np[i, 1, EX_V:EX_V + 1024] = sh_msa @ sv.T

        soT = np.asarray(inputs["soW"], f32)[i].T * g_msa[None, :]  # gate folded
        wso[i] = lhst5(soT, DCH)
        exw_np[i, 0, EX_SO:EX_SO + 1024] = g_msa * sob

        cqT = np.asarray(inputs["cqW"], f32)[i].T
        wcq[i] = lhst5(cqT, DCH)
        exw_np[i, 0, EX_CQ:EX_CQ + 1024] = -cqT.sum(0)

        coT = np.asarray(inputs["coW"], f32)[i].T
        wco[i] = lhst5(coT, DCH)
        exw_np[i, 0, EX_CO:EX_CO + 1024] = cob

        f1T = np.asarray(inputs["f1W"], f32)[i].T * A_mlp[:, None]
        wf1[i] = lhst5(f1T, FCH)
        exw_np[i, 0, EX_F1:EX_F1 + FCH * P] = -f1T.sum(0)
        exw_np[i, 1, EX_F1:EX_F1 + FCH * P] = sh_mlp @ np.asarray(
            inputs["f1W"], f32)[i].T + f1b

        f2T = np.asarray(inputs["f2W"], f32)[i].T * g_mlp[None, :]  # [DFF, D]
        wf2[i] = np.ascontiguousarray(
            f2T.reshape(4, DCH, P, DCH, P).transpose(3, 0, 2, 1, 4)).astype(BF)
        exw_np[i, 0, EX_F2:EX_F2 + 1024] = g_mlp * f2b

        ckT = np.asarray(inputs["ckW"], f32)[i].T
        cvT = np.asarray(inputs["cvW"], f32)[i].T
        wck_f[i] = lhst5(ckT, DCH)
        wcv_f[i] = np.ascontiguousarray(
            cvT.reshape(DCH, P, 2, 512).transpose(2, 1, 0, 3)).astype(BF)

    gath_np = np.zeros((P, 256), f32)
    for b in range(16):
        gath_np[64, 16 * b + b] = 1.0
    bc_np = np.zeros((16, DCH * P), f32)
    for j in range(DCH):
        bc_np[2 * j, j * P:j * P + 64] = 1.0
        bc_np[2 * j + 1, j * P + 64:(j + 1) * P] = 1.0

    ctx_t = sbufize(ctx.T).astype(BF)
    shared = dict(ctx_t=ctx_t, gath=gath_np.astype(BF), bcast=bc_np.astype(BF),
                  cos2=np.ascontiguousarray(cos2),
                  ss2=np.ascontiguousarray(ss2), permT=permT.astype(BF),
                  exw=exw_np.astype(BF),
                  wqk=wqk, wv=wvv, wso=wso, wcq=wcq, wck=wck_f, wcv=wcv_f,
                  wco=wco, wf1=wf1, wf2=wf2)
    in_maps = []
    for c in range(NC):
        m = dict(shared)
        m["x_t"] = sbufize(np.ascontiguousarray(x.T[:, c * LC:(c + 1) * LC]))
        in_maps.append(m)
    return in_maps


_last = {}


def kernel(**inputs):
    import os
    if "nc" not in _cache:
        _cache["nc"] = _build()
    nc = _cache["nc"]
    in_maps = _host_prep(inputs)
    trace = bool(os.environ.get("KERNEL_TRACE"))
    res = run_bass_kernel_spmd(nc, in_maps, core_ids=list(range(NC)), trace=trace)
    _last["res"] = res
    outs = []
    for c in range(NC):
        o = res.results[c]["out_t"]  # [128, DCH*LC]
        outs.append(o.reshape(P, DCH, LC).transpose(1, 0, 2).reshape(D, LC))
    xT = np.concatenate(outs, axis=1)  # [D, L]
    return np.ascontiguousarray(xT.T)[None].astype(np.float32)


# revision 35
# speedup vs baseline: 1.0171x; 1.0171x over previous
"""DiT 2-block forward on 8 trn2 NeuronCores.

Strategy: sequence-parallel matmuls (each core owns 384 tokens, full weights,
channel-major activations) + head-parallel self-attention (2 heads/core) with
2 AllToAll exchanges per block. Cross-attention is fully local (replicated
ctx K/V computed during the qkv-exchange wait). LayerNorm affine/modulation
and all biases are folded into the weights host-side: the kernel only computes
per-token stats (mean*rsqrt) and applies them via one broadcast multiply on
the input plus one extra 2-row matmul per output tile, accumulated in PSUM.
Attention loops are software-pipelined so the PE never idles behind EXP.
"""
import numpy as np
import ml_dtypes

import concourse.bass as bass
import concourse.mybir as mybir
import concourse.tile as tile
from concourse import bacc
from concourse.bass_utils import run_bass_kernel_spmd

P = 128
L, D, H, HD, S, NB, DFF = 3072, 1024, 16, 64, 512, 2, 4096
NC = 8
LC = L // NC            # 384 tokens per core
DCH = D // P            # 8 din chunks
LCH = L // 512          # 6 l-chunks
MT = L // P             # 24 m-tiles (self)
KTC = S // P            # 4 k-tiles (cross)
FCH = DFF // P          # 32 dff chunks
dt = mybir.dt
AF = mybir.ActivationFunctionType
ALU = mybir.AluOpType
BF = ml_dtypes.bfloat16

# extra-weight (bias/mean-row) column offsets inside exw
EX_QK = 0                # 16 tiles x 128  (row0=-w1, row1=bW)
EX_V = EX_QK + 16 * P    # 2 groups x 512  (row0=-vw1, row1=bWv)
EX_SO = EX_V + 2 * 512   # 8 x 128         (row0=g*sob)
EX_CQ = EX_SO + DCH * P  # 8 x 128         (row0=-w1)
EX_CO = EX_CQ + DCH * P  # 8 x 128         (row0=cob)
EX_F1 = EX_CO + DCH * P  # 32 x 128        (row0=-w1, row1=bW+f1b)
EX_F2 = EX_F1 + FCH * P  # 8 x 128         (row0=g*f2b)
EX_COLS = EX_F2 + DCH * P

_cache = {}


def _build():
    nc = bacc.Bacc("TRN2", target_bir_lowering=False, debug=False,
                   enable_asserts=True, num_devices=NC)

    # ---------------- inputs ----------------
    x_t = nc.dram_tensor("x_t", [P, DCH * LC], dt.float32, kind="ExternalInput")
    ctx_t = nc.dram_tensor("ctx_t", [P, DCH * S], dt.bfloat16, kind="ExternalInput")
    cos2 = nc.dram_tensor("cos2", [P, L], dt.bfloat16, kind="ExternalInput")
    ss2 = nc.dram_tensor("ss2", [P, L], dt.bfloat16, kind="ExternalInput")
    permT = nc.dram_tensor("permT", [P, P], dt.bfloat16, kind="ExternalInput")
    exw = nc.dram_tensor("exw", [NB, 2, EX_COLS], dt.bfloat16, kind="ExternalInput")
    wqk = nc.dram_tensor("wqk", [NB, 16, P, DCH, P], dt.bfloat16, kind="ExternalInput")
    wv = nc.dram_tensor("wv", [NB, 2, P, DCH, 512], dt.bfloat16, kind="ExternalInput")
    wso = nc.dram_tensor("wso", [NB, DCH, P, DCH, P], dt.bfloat16, kind="ExternalInput")
    wcq = nc.dram_tensor("wcq", [NB, DCH, P, DCH, P], dt.bfloat16, kind="ExternalInput")
    wck = nc.dram_tensor("wck", [NB, DCH, P, DCH, P], dt.bfloat16, kind="ExternalInput")
    wcv = nc.dram_tensor("wcv", [NB, 2, P, DCH, 512], dt.bfloat16, kind="ExternalInput")
    wco = nc.dram_tensor("wco", [NB, DCH, P, DCH, P], dt.bfloat16, kind="ExternalInput")
    wf1 = nc.dram_tensor("wf1", [NB, FCH, P, DCH, P], dt.bfloat16, kind="ExternalInput")
    wf2 = nc.dram_tensor("wf2", [NB, DCH, 4, P, DCH, P], dt.bfloat16, kind="ExternalInput")
    gath = nc.dram_tensor("gath", [P, 256], dt.bfloat16, kind="ExternalInput")
    bcast = nc.dram_tensor("bcast", [16, DCH * P], dt.bfloat16, kind="ExternalInput")
    out_t = nc.dram_tensor("out_t", [P, DCH * LC], dt.float32, kind="ExternalOutput")

    RG = [list(range(NC))]

    from contextlib import ExitStack
    with tile.TileContext(nc) as tc, ExitStack() as ctx:
        cpool = ctx.enter_context(tc.tile_pool(name="cpool", bufs=1))
        spool = ctx.enter_context(tc.tile_pool(name="spool", bufs=1))
        wpool = ctx.enter_context(tc.tile_pool(name="wpool", bufs=6))
        wvpool = ctx.enter_context(tc.tile_pool(name="wvpool", bufs=2))
        ppool = ctx.enter_context(tc.tile_pool(name="ppool", bufs=6))
        opool = ctx.enter_context(tc.tile_pool(name="opool", bufs=2))
        bigp = ctx.enter_context(tc.tile_pool(name="bigp", bufs=1))
        stg = ctx.enter_context(tc.tile_pool(name="stg", bufs=2))
        stg1 = ctx.enter_context(tc.tile_pool(name="stg1", bufs=1))
        psA = ctx.enter_context(tc.tile_pool(name="psA", bufs=2, space="PSUM"))
        psB = ctx.enter_context(tc.tile_pool(name="psB", bufs=4, space="PSUM"))
        dram = ctx.enter_context(tc.tile_pool(name="dram", bufs=1, space="DRAM"))

        # ---------- persistent tiles ----------
        x_sb = cpool.tile([P, DCH, LC], dt.float32, tag="x_sb")
        nc.sync.dma_start(x_sb[:], x_t.ap().rearrange("p (o t) -> p o t", t=LC))
        ctxb = cpool.tile([P, DCH, S], dt.bfloat16, tag="ctxb")
        nc.sync.dma_start(ctxb[:], ctx_t.ap().rearrange("p (o t) -> p o t", t=S))
        cos_sb = cpool.tile([P, L], dt.bfloat16, tag="cos_sb")
        nc.sync.dma_start(cos_sb[:], cos2.ap())
        ss_sb = cpool.tile([P, L], dt.bfloat16, tag="ss_sb")
        nc.sync.dma_start(ss_sb[:], ss2.ap())
        pT_sb = cpool.tile([P, P], dt.bfloat16, tag="pT_sb")
        nc.sync.dma_start(pT_sb[:], permT.ap())
        ones1 = cpool.tile([P, 1], dt.bfloat16, tag="ones1")
        nc.gpsimd.memset(ones1[:], 1.0)
        onesr = cpool.tile([1, P], dt.bfloat16, tag="onesr")
        nc.gpsimd.memset(onesr[:], 1.0)
        onestok = cpool.tile([1, LC], dt.bfloat16, tag="onestok")
        nc.gpsimd.memset(onestok[:], 1.0)
        epsb = cpool.tile([P, 1], dt.float32, tag="epsb")
        nc.gpsimd.memset(epsb[:], 1e-6)
        gath_sb = cpool.tile([P, 256], dt.bfloat16, tag="gath_sb")
        nc.sync.dma_start(gath_sb[:], gath.ap())
        bc_sb = cpool.tile([16, DCH * P], dt.bfloat16, tag="bc_sb")
        nc.sync.dma_start(bc_sb[:], bcast.ap())

        # ---------- LN stats: rsb (rs bcast to 128p) + rhs2 ([mrs; ones]) + xr ----------
        def ln_begin(nm):
            return dict(
                nm=nm,
                xb=spool.tile([P, DCH, LC], dt.bfloat16, tag="xb", name=f"xb_{nm}"),
                ps1=psB.tile([P, 512], dt.float32, tag="psb", name=f"ps1_{nm}"),
                ps2=psB.tile([P, 512], dt.float32, tag="psb", name=f"ps2_{nm}"))

        def ln_chunk(st, o):
            xb = st['xb']
            if o % 2 == 0:
                nc.vector.tensor_copy(xb[:, o, :], x_sb[:, o, :])
            else:
                nc.scalar.activation(xb[:, o, :], x_sb[:, o, :], AF.Copy)
            xsq = stg.tile([P, LC], dt.bfloat16, tag="lnsq", name=f"xsq_{st['nm']}{o}")
            nc.vector.tensor_tensor(xsq[:], xb[:, o, :], xb[:, o, :], ALU.mult)
            nc.tensor.matmul(st['ps1'][:1, :LC], ones1[:], xb[:, o, :],
                             start=(o == 0), stop=(o == DCH - 1))
            nc.tensor.matmul(st['ps2'][:1, :LC], ones1[:], xsq[:],
                             start=(o == 0), stop=(o == DCH - 1))

        def ln_finish(st):
            xb, ps1, ps2 = st['xb'], st['ps1'], st['ps2']
            mrow = spool.tile([1, LC], dt.float32, tag="mrow")
            nc.vector.tensor_scalar_mul(mrow[:], ps1[:1, :LC], 1.0 / D)
            msq = spool.tile([1, LC], dt.float32, tag="msq")
            nc.vector.tensor_tensor(msq[:], mrow[:], mrow[:], ALU.mult)
            varr = spool.tile([1, LC], dt.float32, tag="varr")
            nc.vector.scalar_tensor_tensor(varr[:], ps2[:1, :LC], 1.0 / D, msq[:],
                                           ALU.mult, ALU.subtract)
            rs = spool.tile([1, LC], dt.float32, tag="rs")
            nc.scalar.activation(rs[:], varr[:], AF.Abs_reciprocal_sqrt, bias=epsb[:1])
            rsmb = spool.tile([1, LC], dt.bfloat16, tag="rsmb")
            nc.vector.tensor_copy(rsmb[:], rs[:])
            rhs2 = stg.tile([2, LC], dt.bfloat16, tag="rhs2")
            nc.gpsimd.memset(rhs2[:], 1.0)
            nc.vector.tensor_tensor(rhs2[0:1, :], mrow[:], rs[:], ALU.mult)
            # broadcast rs to 128 partitions via K=1 matmul
            psbc = psB.tile([P, 512], dt.float32, tag="psb")
            nc.tensor.matmul(psbc[:, 0:LC], onesr[:], rsmb[:], start=True, stop=True)
            rsb = stg.tile([P, LC], dt.bfloat16, tag="rsb")
            nc.vector.tensor_copy(rsb[:], psbc[:, 0:LC])
            xr = spool.tile([P, DCH, LC], dt.bfloat16, tag="hx",
                            name=f"xr_{st['nm']}")
            nc.vector.tensor_tensor(xr[:, 0:5, :], xb[:, 0:5, :],
                                    rsb[:, None, :].to_broadcast([P, 5, LC]), ALU.mult)
            nc.gpsimd.tensor_tensor(xr[:, 5:8, :], xb[:, 5:8, :],
                                    rsb[:, None, :].to_broadcast([P, 3, LC]), ALU.mult)
            return xr, rhs2

        def emit_ln(nm):
            st = ln_begin(nm)
            xb = st['xb']
            nc.vector.tensor_copy(xb[:, 0:5, :], x_sb[:, 0:5, :])
            nc.scalar.activation(xb[:, 5:8, :], x_sb[:, 5:8, :], AF.Copy)
            for o in range(DCH):
                nc.tensor.matmul(st['ps1'][:1, :LC], ones1[:], xb[:, o, :],
                                 start=(o == 0), stop=(o == DCH - 1))
            xsq = spool.tile([P, DCH, LC], dt.bfloat16, tag="lnsq",
                             name=f"xsqf_{nm}")
            nc.vector.tensor_tensor(xsq[:], xb[:], xb[:], ALU.mult)
            for o in range(DCH):
                nc.tensor.matmul(st['ps2'][:1, :LC], ones1[:], xsq[:, o, :],
                                 start=(o == 0), stop=(o == DCH - 1))
            return ln_finish(st)

        # ---------- blocks ----------
        for i in range(NB):
            def exload(off, n):  # load [2, n] extra-weight slice for this phase
                exch = stg1.tile([2, 16 * P], dt.bfloat16, tag="exch")
                nc.sync.dma_start(exch[:, 0:n], exw.ap()[i, :, off:off + n])
                return exch

            # ===== AdaLN + self-attention =====
            xr, rhs2 = emit_ln(f"ln1b{i}")

            qkv_in = dram.tile([NC, 2 * P * LC], dt.bfloat16, tag="qkv_in")
            qkv_out = dram.tile([NC, 2 * P * LC], dt.bfloat16, tag="qkv_out")
            v_in = dram.tile([NC, P * LC], dt.bfloat16, tag="v_in")
            v_out = dram.tile([NC, P * LC], dt.bfloat16, tag="v_out")
            # q, k projections (channel-major lhsT tiles) -> staged to DRAM
            stq = stg1.tile([P, DCH, LC], dt.bfloat16, tag="stq")
            stk = stg1.tile([P, DCH, LC], dt.bfloat16, tag="stk")
            exqk = exload(EX_QK, 16 * P)
            for j in range(16):
                wt = wpool.tile([P, DCH, P], dt.bfloat16, tag="w8")
                nc.sync.dma_start(wt[:], wqk.ap()[i, j])
                pp = psB.tile([P, 512], dt.float32, tag="psb")
                for o in range(DCH):
                    nc.tensor.matmul(pp[:, :LC], wt[:, o, :], xr[:, o, :],
                                     start=(o == 0), stop=False)
                nc.tensor.matmul(pp[:, :LC], exqk[:, j * P:(j + 1) * P], rhs2[:],
                                 start=False, stop=True)
                dst = (stq if j < 8 else stk)[:, j % 8, :]
                if j % 2 == 0:
                    nc.vector.tensor_copy(dst, pp[:, :LC])
                else:
                    nc.scalar.activation(dst, pp[:, :LC], AF.Copy)
            nc.sync.dma_start(
                qkv_in[:, 0:P * LC].rearrange("s (p t) -> p s t", p=P), stq[:])
            nc.sync.dma_start(
                qkv_in[:, P * LC:2 * P * LC].rearrange("s (p t) -> p s t", p=P), stk[:])
            nc.gpsimd.collective_compute("AllToAll", ALU.bypass, replica_groups=RG,
                                         ins=[qkv_in.opt()], outs=[qkv_out.opt()])
            # v projection (token-major)
            stv = stg1.tile([P, 2, 3, 512], dt.bfloat16, tag="stv")
            exv = exload(EX_V, 2 * 512)
            for g in range(2):
                wvt = wvpool.tile([P, DCH, 512], dt.bfloat16, tag="wv512")
                nc.sync.dma_start(wvt[:], wv.ap()[i, g])
                for tcix in range(LC // P):
                    pp = psB.tile([P, 512], dt.float32, tag="psb")
                    for o in range(DCH):
                        nc.tensor.matmul(pp[:], xr[:, o, tcix * P:(tcix + 1) * P],
                                         wvt[:, o, :], start=(o == 0), stop=False)
                    nc.tensor.matmul(pp[:], rhs2[:, tcix * P:(tcix + 1) * P],
                                     exv[:, g * 512:(g + 1) * 512],
                                     start=False, stop=True)
                    if tcix % 2 == 0:
                        nc.vector.tensor_copy(stv[:, g, tcix, :], pp[:])
                    else:
                        nc.scalar.activation(stv[:, g, tcix, :], pp[:], AF.Copy)
            for g in range(2):
                for s4 in range(4):
                    dstv = v_in[4 * g + s4, :] \
                        .rearrange("(tc p c) -> p tc c", p=P, c=P)
                    nc.sync.dma_start(dstv, stv[:, g, :, s4 * P:(s4 + 1) * P])
            nc.gpsimd.collective_compute("AllToAll", ALU.bypass, replica_groups=RG,
                                         ins=[v_in.opt()], outs=[v_out.opt()])

            # ===== cross K/V over all ctx tokens (fills the AllToAll wait) =====
            k2c = bigp.tile([P, DCH, S], dt.bfloat16, tag="k2c")

            def emit_crossk(jlist):
                for j in jlist:
                    wt = wpool.tile([P, DCH, P], dt.bfloat16, tag="w8", name=f"wck{j}")
                    nc.sync.dma_start(wt[:], wck.ap()[i, j])
                    psk = psB.tile([P, 512], dt.float32, tag="psb", name=f"psk{j}")
                    for o in range(DCH):
                        nc.tensor.matmul(psk[:, :S], wt[:, o, :], ctxb[:, o, :],
                                         start=(o == 0), stop=(o == DCH - 1))
                    if j % 2 == 0:
                        nc.vector.tensor_copy(k2c[:, j, :], psk[:, :S])
                    else:
                        nc.scalar.activation(k2c[:, j, :], psk[:, :S], AF.Copy)

            emit_crossk(range(0, 4))
            # cross-V is emitted as PE filler work inside the flash loop below,
            # to keep the PE saturated (and the HAM clock warm) while EXP gates.
            vextc = bigp.tile([P, KTC, DCH, 130], dt.bfloat16, tag="vextc")
            nc.gpsimd.memset(vextc[:, :, :, 64:65], 1.0)
            nc.gpsimd.memset(vextc[:, :, :, 129:130], 1.0)
            fillers = []
            cvstate = {}

            def _cv_load(half):
                wcvt = wvpool.tile([P, DCH, 512], dt.bfloat16, tag="wv512",
                                   name=f"wcvt{half}")
                nc.sync.dma_start(wcvt[:], wcv.ap()[i, half])
                cvstate['w'] = wcvt

            def _cv_mm(half, kt, o):
                if o == 0:
                    cvstate['psv'] = psB.tile([P, 512], dt.float32, tag="psb",
                                              name=f"psv{half}_{kt}")
                nc.tensor.matmul(cvstate['psv'][:], ctxb[:, o, kt * P:(kt + 1) * P],
                                 cvstate['w'][:, o, :],
                                 start=(o == 0), stop=(o == DCH - 1))

            def _cv_copy(half, kt):
                pv3 = cvstate['psv'][:].rearrange("p (j c) -> p j c", c=P)
                jj = slice(4 * half, 4 * half + 4)
                nc.vector.tensor_copy(vextc[:, kt, jj, 0:64], pv3[:, :, 0:64])
                nc.vector.tensor_copy(vextc[:, kt, jj, 65:129], pv3[:, :, 64:128])

            from functools import partial
            for half in range(2):
                fillers.append(partial(_cv_load, half))
                for kt in range(KTC):
                    for o in range(DCH):
                        fillers.append(partial(_cv_mm, half, kt, o))
                    fillers.append(partial(_cv_copy, half, kt))
            fillers.reverse()  # pop() from the front

            # ===== unpack qkv exchange =====
            q_sb = bigp.tile([P, L], dt.bfloat16, tag="q_sb")
            k_sb = bigp.tile([P, L], dt.bfloat16, tag="k_sb")
            nc.sync.dma_start(q_sb[:].rearrange("p (s t) -> p s t", t=LC),
                              qkv_out[:, 0:P * LC].rearrange("s (p t) -> p s t", p=P))
            nc.sync.dma_start(k_sb[:].rearrange("p (s t) -> p s t", t=LC),
                              qkv_out[:, P * LC:2 * P * LC].rearrange(
                                  "s (p t) -> p s t", p=P))
            vext = bigp.tile([P, MT, 130], dt.bfloat16, tag="vext")
            nc.gpsimd.memset(vext[:, :, 64:65], 1.0)
            nc.gpsimd.memset(vext[:, :, 129:130], 1.0)
            for s in range(NC):
                vsrc = v_out[s, :].rearrange("(tc p c) -> p tc c", p=P, c=P)
                nc.sync.dma_start(vext[:, s * 3:(s + 1) * 3, 0:64], vsrc[:, :, 0:64])
                nc.sync.dma_start(vext[:, s * 3:(s + 1) * 3, 65:129],
                                  vsrc[:, :, 64:128])

            # RoPE (perm matmul + combine, in-place on q_sb/k_sb; k first)
            for src in (k_sb, q_sb):
                for lc in range(LCH):
                    sl = slice(lc * 512, (lc + 1) * 512)
                    psr = psB.tile([P, 512], dt.float32, tag="psb")
                    nc.tensor.matmul(psr[:], pT_sb[:], src[:, sl], start=True, stop=True)
                    rt = stg.tile([P, 512], dt.bfloat16, tag="rtmp")
                    nc.vector.tensor_tensor(rt[:], psr[:], ss_sb[:, sl], ALU.mult)
                    nc.gpsimd.tensor_tensor(src[:, sl], src[:, sl], cos_sb[:, sl],
                                            ALU.mult)
                    nc.vector.tensor_tensor(src[:, sl], src[:, sl], rt[:], ALU.add)
            qr, kr = q_sb, k_sb

            # flash attention, software-pipelined: scores(mt+1) issued before PV(mt)
            o_h0 = opool.tile([65, L], dt.bfloat16, tag="osb")
            o_h1 = opool.tile([65, L], dt.bfloat16, tag="osb")
            for lc in range(LCH):
                sl = slice(lc * 512, (lc + 1) * 512)
                pso0 = psB.tile([P, 512], dt.float32, tag="psb")
                pso1 = psB.tile([P, 512], dt.float32, tag="psb")

                def sc_mm(mt):
                    pqk = psA.tile([P, 1024], dt.float32, tag="psa")
                    nc.tensor.matmul(pqk[:, 0:512], kr[0:64, mt * P:(mt + 1) * P],
                                     qr[0:64, sl], start=True, stop=True)
                    nc.tensor.matmul(pqk[:, 512:1024], kr[64:128, mt * P:(mt + 1) * P],
                                     qr[64:128, sl], start=True, stop=True)
                    return pqk

                pqk_cur = sc_mm(0)
                for mt in range(MT):
                    Pt = ppool.tile([P, 1024], dt.bfloat16, tag="Pt")
                    nc.scalar.activation(Pt[:], pqk_cur[:], AF.Exp, scale=HD ** -0.5)
                    if mt + 1 < MT:
                        pqk_cur = sc_mm(mt + 1)
                    nc.tensor.matmul(pso0[:65, :], vext[:, mt, 0:65], Pt[:, 0:512],
                                     start=(mt == 0), stop=(mt == MT - 1))
                    nc.tensor.matmul(pso1[:65, :], vext[:, mt, 65:130], Pt[:, 512:1024],
                                     start=(mt == 0), stop=(mt == MT - 1))
                    if fillers:
                        fillers.pop()()
                nc.vector.tensor_copy(o_h0[:, sl], pso0[:65, :])
                nc.vector.tensor_copy(o_h1[:, sl], pso1[:65, :])
            while fillers:
                fillers.pop()()

            # ===== o exchange + o-proj + residual =====
            oa_in = dram.tile([NC, 2, 65, LC], dt.bfloat16, tag="oa_in")
            oa_out = dram.tile([NC, 2, 65, LC], dt.bfloat16, tag="oa_out")
            nc.sync.dma_start(oa_in[:, 0].rearrange("s c t -> c s t"),
                              o_h0[:].rearrange("c (s t) -> c s t", t=LC))
            nc.sync.dma_start(oa_in[:, 1].rearrange("s c t -> c s t"),
                              o_h1[:].rearrange("c (s t) -> c s t", t=LC))
            nc.gpsimd.collective_compute("AllToAll", ALU.bypass, replica_groups=RG,
                                         ins=[oa_in.opt()], outs=[oa_out.opt()])
            emit_crossk(range(4, DCH))

            orecv = spool.tile([P, DCH, LC], dt.bfloat16, tag="xb")
            nc.sync.dma_start(orecv[0:64, :, :],
                              oa_out[:, 0, 0:64, :].rearrange("s c t -> c s t"))
            nc.sync.dma_start(orecv[64:128, :, :],
                              oa_out[:, 1, 0:64, :].rearrange("s c t -> c s t"))
            sums = spool.tile([16, LC], dt.bfloat16, tag="sums")
            nc.sync.dma_start(sums[:], oa_out[:, :, 64, :])
            rcp = spool.tile([16, LC], dt.float32, tag="rcp")
            nc.vector.reciprocal(rcp[:], sums[:])
            rcpb = spool.tile([16, LC], dt.bfloat16, tag="rcpb")
            nc.vector.tensor_copy(rcpb[:], rcp[:])
            exso = exload(EX_SO, DCH * P)
            for j in range(DCH):
                psbj = psB.tile([P, 512], dt.float32, tag="psb")
                nc.tensor.matmul(psbj[:, 0:LC], bc_sb[:, j * P:(j + 1) * P], rcpb[:],
                                 start=True, stop=True)
                nc.vector.tensor_tensor(orecv[:, j, :], orecv[:, j, :],
                                        psbj[:, 0:LC], ALU.mult)
            for j in range(DCH):
                wt = wpool.tile([P, DCH, P], dt.bfloat16, tag="w8")
                nc.sync.dma_start(wt[:], wso.ap()[i, j])
                pp = psB.tile([P, 512], dt.float32, tag="psb")
                for o in range(DCH):
                    nc.tensor.matmul(pp[:, :LC], wt[:, o, :], orecv[:, o, :],
                                     start=(o == 0), stop=False)
                nc.tensor.matmul(pp[:, :LC], exso[0:1, j * P:(j + 1) * P], onestok[:],
                                 start=False, stop=True)
                nc.vector.scalar_tensor_tensor(x_sb[:, j, :], pp[:, :LC], 0.0,
                                               x_sb[:, j, :], ALU.bypass, ALU.add)

            # ===== cross-attention (fully local) =====
            xr2, rhs2c = emit_ln(f"ln2b{i}")
            cq_sb = spool.tile([P, DCH, LC], dt.bfloat16, tag="xb")
            excq = exload(EX_CQ, DCH * P)

            def emit_cq(j):
                wt = wpool.tile([P, DCH, P], dt.bfloat16, tag="w8", name=f"wcq{j}")
                nc.sync.dma_start(wt[:], wcq.ap()[i, j])
                pp = psB.tile([P, 512], dt.float32, tag="psb", name=f"cqpp{j}")
                for o in range(DCH):
                    nc.tensor.matmul(pp[:, :LC], wt[:, o, :], xr2[:, o, :],
                                     start=(o == 0), stop=False)
                nc.tensor.matmul(pp[:, :LC], excq[0:1, j * P:(j + 1) * P],
                                 rhs2c[0:1, :], start=False, stop=True)
                nc.vector.tensor_copy(cq_sb[:, j, :], pp[:, :LC])

            emit_cq(0)
            emit_cq(1)

            o_all = spool.tile([P, DCH, LC], dt.bfloat16, tag="lnscratch")
            oe65 = stg1.tile([P, DCH, LC], dt.bfloat16, tag="stk")
            oo65 = stg1.tile([P, DCH, LC], dt.bfloat16, tag="stq")
            psden = psB.tile([P, 512], dt.float32, tag="psb")
            for j in range(DCH):
                psca = psB.tile([P, 512], dt.float32, tag="psb")
                pscb = psB.tile([P, 512], dt.float32, tag="psb")

                def csc_mm(kt):
                    pqc = psA.tile([P, 1024], dt.float32, tag="psa")
                    nc.tensor.matmul(pqc[:, 0:LC], k2c[0:64, j, kt * P:(kt + 1) * P],
                                     cq_sb[0:64, j, :], start=True, stop=True)
                    nc.tensor.matmul(pqc[:, 512:512 + LC],
                                     k2c[64:128, j, kt * P:(kt + 1) * P],
                                     cq_sb[64:128, j, :], start=True, stop=True)
                    return pqc

                pqc_cur = csc_mm(0)
                if j + 2 < DCH:
                    emit_cq(j + 2)
                for kt in range(KTC):
                    Ptc = ppool.tile([P, 1024], dt.bfloat16, tag="Pt")
                    nc.scalar.activation(
                        Ptc[:, 0:2 * LC].rearrange("p (b c) -> p b c", c=LC),
                        pqc_cur[:].rearrange("p (b c) -> p b c", c=512)[:, :, 0:LC],
                        AF.Exp, scale=HD ** -0.5)
                    if kt + 1 < KTC:
                        pqc_cur = csc_mm(kt + 1)
                    nc.tensor.matmul(psca[:65, :LC], vextc[:, kt, j, 0:65],
                                     Ptc[:, 0:LC], start=(kt == 0), stop=(kt == KTC - 1))
                    nc.tensor.matmul(pscb[:65, :LC], vextc[:, kt, j, 65:130],
                                     Ptc[:, LC:2 * LC], start=(kt == 0),
                                     stop=(kt == KTC - 1))
                nc.vector.tensor_copy(oe65[0:65, j, :], psca[0:65, :LC])
                nc.scalar.activation(oo65[0:65, j, :], pscb[0:65, :LC], AF.Copy)
                # gather the two denominator rows (row 64) into psden rows 2j/2j+1
                nc.tensor.matmul(psden[:16, :LC], gath_sb[0:65, 32 * j:32 * j + 16],
                                 oe65[0:65, j, :], start=(j == 0), stop=False)
                nc.tensor.matmul(psden[:16, :LC], gath_sb[0:65, 32 * j + 16:32 * j + 32],
                                 oo65[0:65, j, :], start=False, stop=(j == DCH - 1))
            # assemble channel-major o via SBUF->SBUF DMAs
            nc.sync.dma_start(o_all[0:64, :, :], oe65[0:64, :, :])
            nc.sync.dma_start(o_all[64:128, :, :], oo65[0:64, :, :])
            rcpc = spool.tile([16, LC], dt.float32, tag="rcp")
            nc.vector.reciprocal(rcpc[:], psden[:16, :LC])
            rcpcb = spool.tile([16, LC], dt.bfloat16, tag="rcpb")
            nc.vector.tensor_copy(rcpcb[:], rcpc[:])
            for j in range(DCH):
                psbj = psB.tile([P, 512], dt.float32, tag="psb")
                nc.tensor.matmul(psbj[:, 0:LC], bc_sb[:, j * P:(j + 1) * P], rcpcb[:],
                                 start=True, stop=True)
                nc.vector.tensor_tensor(o_all[:, j, :], o_all[:, j, :],
                                        psbj[:, 0:LC], ALU.mult)
            exco = exload(EX_CO, DCH * P)
            for j in range(DCH):
                wt = wpool.tile([P, DCH, P], dt.bfloat16, tag="w8")
                nc.sync.dma_start(wt[:], wco.ap()[i, j])
                pp = psB.tile([P, 512], dt.float32, tag="psb")
                for o in range(DCH):
                    nc.tensor.matmul(pp[:, :LC], wt[:, o, :], o_all[:, o, :],
                                     start=(o == 0), stop=False)
                nc.tensor.matmul(pp[:, :LC], exco[0:1, j * P:(j + 1) * P], onestok[:],
                                 start=False, stop=True)
                nc.vector.scalar_tensor_tensor(x_sb[:, j, :], pp[:, :LC], 0.0,
                                               x_sb[:, j, :], ALU.bypass, ALU.add)

            # ===== AdaLN + MLP =====
            xr3, rhs2m = emit_ln(f"ln3b{i}")
            g_sb = cpool.tile([P, FCH, LC], dt.bfloat16, tag="g_sb")
            exf1 = exload(EX_F1, 16 * P)
            for j in range(FCH):
                if j == 16:
                    exf1 = exload(EX_F1 + 16 * P, 16 * P)
                wt = wpool.tile([P, DCH, P], dt.bfloat16, tag="w8")
                nc.sync.dma_start(wt[:], wf1.ap()[i, j])
                pp = psB.tile([P, 512], dt.float32, tag="psb")
                for o in range(DCH):
                    nc.tensor.matmul(pp[:, :LC], wt[:, o, :], xr3[:, o, :],
                                     start=(o == 0), stop=False)
                nc.tensor.matmul(pp[:, :LC], exf1[:, (j % 16) * P:(j % 16 + 1) * P],
                                 rhs2m[:], start=False, stop=True)
                nc.scalar.activation(g_sb[:, j, :], pp[:, :LC], AF.Gelu)
            exf2 = exload(EX_F2, DCH * P)
            for j in range(DCH):
                pp = psB.tile([P, 512], dt.float32, tag="psb")
                for og in range(4):
                    wt = wpool.tile([P, DCH, P], dt.bfloat16, tag="w8")
                    nc.sync.dma_start(wt[:], wf2.ap()[i, j, og])
                    for o2 in range(DCH):
                        nc.tensor.matmul(pp[:, :LC], wt[:, o2, :], g_sb[:, og * DCH + o2, :],
                                         start=(og == 0 and o2 == 0), stop=False)
                nc.tensor.matmul(pp[:, :LC], exf2[0:1, j * P:(j + 1) * P], onestok[:],
                                 start=False, stop=True)
                nc.vector.scalar_tensor_tensor(x_sb[:, j, :], pp[:, :LC], 0.0,
                                               x_sb[:, j, :], ALU.bypass, ALU.add)

        nc.sync.dma_start(out_t.ap().rearrange("p (o t) -> p o t", t=LC), x_sb[:])

    nc.compile()
    return nc


def _host_prep(inputs):
    """Build per-core in_maps from full inputs (LN affine folded into weights)."""
    f32 = np.float32
    x = np.asarray(inputs["x"], f32)[0]           # [L, D]
    te = np.asarray(inputs["timestep_emb"], f32)  # [1, D]
    ctx = np.asarray(inputs["context_emb"], f32)[0]
    rope = np.asarray(inputs["rope_emb"], f32)    # [L, HD]
    cos, sin = np.cos(rope), np.sin(rope)

    def sbufize(a2d):  # [D, T] -> [128, DCH*T] channel-major sbuf layout
        Dd, T = a2d.shape
        return np.ascontiguousarray(
            a2d.reshape(Dd // P, P, T).transpose(1, 0, 2).reshape(P, (Dd // P) * T))

    def lhst5(WT, jn):  # WT [D, DOUT] -> [jn, P, DCH, P] tiles of W^T
        Dd, DO = WT.shape
        a = WT.reshape(DCH, P, jn, P)         # [o, p, j, f]
        return np.ascontiguousarray(a.transpose(2, 1, 0, 3)).astype(BF)  # [j, p, o, f]

    cos2 = np.tile(cos.T, (2, 1)).astype(BF)                      # [128, L]
    ssg = np.concatenate([-sin.T[:32], sin.T[32:]], 0)
    ss2 = np.tile(ssg, (2, 1)).astype(BF)
    sig = (np.arange(P) + 32) % 64 + 64 * (np.arange(P) // 64)
    permT = np.zeros((P, P), f32)
    permT[sig, np.arange(P)] = 1.0

    wqk = np.zeros((NB, 16, P, DCH, P), BF)
    wvv = np.zeros((NB, 2, P, DCH, 512), BF)
    wso = np.zeros((NB, DCH, P, DCH, P), BF)
    wcq = np.zeros((NB, DCH, P, DCH, P), BF)
    wck_f = np.zeros((NB, DCH, P, DCH, P), BF)
    wcv_f = np.zeros((NB, 2, P, DCH, 512), BF)
    wco = np.zeros((NB, DCH, P, DCH, P), BF)
    wf1 = np.zeros((NB, FCH, P, DCH, P), BF)
    wf2 = np.zeros((NB, DCH, 4, P, DCH, P), BF)
    exw_np = np.zeros((NB, 2, EX_COLS), f32)

    for i in range(NB):
        mods = (te @ np.asarray(inputs["adaW"], f32)[i].T
                + np.asarray(inputs["adab"], f32)[i])[0]
        sh_msa, sc_msa, g_msa, sh_mlp, sc_mlp, g_mlp = np.split(mods, 6)
        A_msa, A_mlp = 1.0 + sc_msa, 1.0 + sc_mlp
        sob = np.asarray(inputs["sob"], f32)[i]
        cob = np.asarray(inputs["cob"], f32)[i]
        f1b = np.asarray(inputs["f1b"], f32)[i]
        f2b = np.asarray(inputs["f2b"], f32)[i]

        sq, sk, sv = (np.asarray(inputs[k], f32)[i] for k in ("sqW", "skW", "svW"))
        # folded transposed weights [din, dout]
        sqT = sq.T * A_msa[:, None]
        skT = sk.T * A_msa[:, None]
        svT = sv.T * A_msa[:, None]
        wqk[i, :8] = lhst5(sqT, 8)
        wqk[i, 8:] = lhst5(skT, 8)
        wvv[i] = np.ascontiguousarray(
            svT.reshape(DCH, P, 2, 512).transpose(2, 1, 0, 3)).astype(BF)
        # extra rows: qk (row0=-colsum, row1=sh@W.T)
        exw_np[i, 0, EX_QK:EX_QK + 8 * P] = -sqT.sum(0)
        exw_np[i, 0, EX_QK + 8 * P:EX_QK + 16 * P] = -skT.sum(0)
        exw_np[i, 1, EX_QK:EX_QK + 8 * P] = sh_msa @ sq.T
        exw_np[i, 1, EX_QK + 8 * P:EX_QK + 16 * P] = sh_msa @ sk.T
        exw_np[i, 0, EX_V:EX_V + 1024] = -svT.sum(0)
        exw_np[i, 1, EX_V:EX_V + 1024] = sh_msa @ sv.T

        soT = np.asarray(inputs["soW"], f32)[i].T * g_msa[None, :]  # gate folded
        wso[i] = lhst5(soT, DCH)
        exw_np[i, 0, EX_SO:EX_SO + 1024] = g_msa * sob

        cqT = np.asarray(inputs["cqW"], f32)[i].T
        wcq[i] = lhst5(cqT, DCH)
        exw_np[i, 0, EX_CQ:EX_CQ + 1024] = -cqT.sum(0)

        coT = np.asarray(inputs["coW"], f32)[i].T
        wco[i] = lhst5(coT, DCH)
        exw_np[i, 0, EX_CO:EX_CO + 1024] = cob

        f1T = np.asarray(inputs["f1W"], f32)[i].T * A_mlp[:, None]
        wf1[i] = lhst5(f1T, FCH)
        exw_np[i, 0, EX_F1:EX_F1 + FCH * P] = -f1T.sum(0)
        exw_np[i, 1, EX_F1:EX_F1 + FCH * P] = sh_mlp @ np.asarray(
            inputs["f1W"], f32)[i].T + f1b

        f2T = np.asarray(inputs["f2W"], f32)[i].T * g_mlp[None, :]  # [DFF, D]
        wf2[i] = np.ascontiguousarray(
            f2T.reshape(4, DCH, P, DCH, P).transpose(3, 0, 2, 1, 4)).astype(BF)
        exw_np[i, 0, EX_F2:EX_F2 + 1024] = g_mlp * f2b

        ckT = np.asarray(inputs["ckW"], f32)[i].T
        cvT = np.asarray(inputs["cvW"], f32)[i].T
        wck_f[i] = lhst5(ckT, DCH)
        wcv_f[i] = np.ascontiguousarray(
            cvT.reshape(DCH, P, 2, 512).transpose(2, 1, 0, 3)).astype(BF)

    gath_np = np.zeros((P, 256), f32)
    for b in range(16):
        gath_np[64, 16 * b + b] = 1.0
    bc_np = np.zeros((16, DCH * P), f32)
    for j in range(DCH):
        bc_np[2 * j, j * P:j * P + 64] = 1.0
        bc_np[2 * j + 1, j * P + 64:(j + 1) * P] = 1.0

    ctx_t = sbufize(ctx.T).astype(BF)
    shared = dict(ctx_t=ctx_t, gath=gath_np.astype(BF), bcast=bc_np.astype(BF),
                  cos2=np.ascontiguousarray(cos2),
                  ss2=np.ascontiguousarray(ss2), permT=permT.astype(BF),
                  exw=exw_np.astype(BF),
                  wqk=wqk, wv=wvv, wso=wso, wcq=wcq, wck=wck_f, wcv=wcv_f,
                  wco=wco, wf1=wf1, wf2=wf2)
    in_maps = []
    for c in range(NC):
        m = dict(shared)
        m["x_t"] = sbufize(np.ascontiguousarray(x.T[:, c * LC:(c + 1) * LC]))
        in_maps.append(m)
    return in_maps


_last = {}


def kernel(**inputs):
    import os
    if "nc" not in _cache:
        _cache["nc"] = _build()
    nc = _cache["nc"]
    in_maps = _host_prep(inputs)
    trace = bool(os.environ.get("KERNEL_TRACE"))
    res = run_bass_kernel_spmd(nc, in_maps, core_ids=list(range(NC)), trace=trace)
    _last["res"] = res
    outs = []
    for c in range(NC):
        o = res.results[c]["out_t"]  # [128, DCH*LC]
        outs.append(o.reshape(P, DCH, LC).transpose(1, 0, 2).reshape(D, LC))
    xT = np.concatenate(outs, axis=1)  # [D, L]
    return np.ascontiguousarray(xT.T)[None].astype(np.float32)
